# revision 5
# baseline (speedup 1.0000x reference)
"""GPT forward (L=4, H=12, E=768, T=1024, B=4, V=50257) on 8 Trainium2 cores.

Sharding: rows (batch x sequence) are zigzag-split across core pairs — core
pair (2b, 2b+1) owns batch b; the even core owns 128-row q-tiles {0,1,6,7},
the odd core {2,3,4,5} (balances causal-attention work). K/V are exchanged
within each pair via AllGather each layer. The tied lm_head is vocab-parallel:
every core computes logits for all 4096 rows over a 6283-wide vocab slice
(final hidden states are AllGathered across all 8 cores). The loss is
assembled on the host from per-core exp-sum partials and the full logits.

All matmuls run in bf16 with fp32 PSUM accumulation. LayerNorm scales/biases
are folded into the adjacent weights on the host. The SPMD program is
identical on every core; per-core differences (row ownership, causal masks,
vocab slice) enter only through input data.
"""

import math

import numpy as np
import ml_dtypes

# ---------------------------------------------------------------- constants
L, H, E, T, B, V = 4, 12, 768, 1024, 4, 50257
D = E // H            # 64
P = 128               # partitions
R = 512               # rows per core (4 q-tiles)
NB = 8                # cores
VP = 6283             # vocab slice per core (8*6283 = 50264 = V+7)
VPAD = NB * VP - V    # 7 zero-padded vocab columns on the last core
NVT = 13              # vocab tiles per core: 12*512 + 139
VT_SIZES = [512] * 12 + [VP - 12 * 512]
ET = E // P           # 6   e-tiles
FT = 4 * E // P       # 24  ffn tiles
OFT = 3 * E // P      # 18  qkv out tiles
NMT = 32              # global row tiles (4096/128)

ZIG = {0: [0, 1, 6, 7], 1: [2, 3, 4, 5]}   # parity -> owned global q-tiles
NKPAD = [3, 4, 7, 8]                       # padded k-extent per q slot
# global k-tile -> (AG block, slot within block); block 0 = even core's kv
PERM = [(0, 0), (0, 1), (1, 0), (1, 1), (1, 2), (1, 3), (0, 2), (0, 3)]

BF16 = ml_dtypes.bfloat16

_CACHE = {}


# ------------------------------------------------------------- wait splitter
def _split_sync_waits(nc, mybir, max_waits=1):
    """Walrus here rejects >1 sync wait per instruction; waits gate issue on
    the in-order engine sequencer, so excess waits move to preceding NoOps."""
    n = [0]

    def fresh():
        n[0] += 1
        return f"I-waitsplit-{n[0]}"

    for fn in nc.m.functions:
        for bb in fn.blocks:
            if not any(
                ins.sync_info and ins.sync_info.on_wait
                and len(ins.sync_info.on_wait) > max_waits
                for ins in bb.instructions
            ):
                continue
            new_insts = []
            for ins in bb.instructions:
                si = ins.sync_info
                if si is not None and si.on_wait and len(si.on_wait) > max_waits:
                    waits = list(si.on_wait)
                    while len(waits) > max_waits:
                        chunk, waits = waits[:max_waits], waits[max_waits:]
                        new_insts.append(mybir.InstNoOp(
                            name=fresh(), opcode="NoOp", engine=ins.engine,
                            ins=[], outs=[],
                            sync_info=mybir.SyncInfo(on_wait=chunk, on_update=[]),
                        ))
                    ins.sync_info = mybir.SyncInfo(
                        on_wait=waits, on_update=list(si.on_update))
                new_insts.append(ins)
            del bb.instructions[:]
            for ins in new_insts:
                bb.add_instruction(ins)
    return n[0]


# ------------------------------------------------------------ program build
def _build_program():
    from contextlib import ExitStack

    import concourse.bass as bass
    import concourse.mybir as mybir
    import concourse.tile as tile
    from concourse.bass import ds
    from concourse.masks import make_identity

    f32 = mybir.dt.float32
    bf16 = mybir.dt.bfloat16
    AF = mybir.ActivationFunctionType
    ALU = mybir.AluOpType

    nc = bass.Bass("TRN2", target_bir_lowering=False, debug=False,
                   num_devices=NB)

    # ---- I/O ----
    x0_h = nc.dram_tensor("x0", [R, E], f32, kind="ExternalInput")
    awT_h = nc.dram_tensor("awT", [L, E, 3 * E], bf16, kind="ExternalInput")
    ab_h = nc.dram_tensor("ab", [L, 3 * E], f32, kind="ExternalInput")
    pwT_h = nc.dram_tensor("pwT", [L, E, E], bf16, kind="ExternalInput")
    pb_h = nc.dram_tensor("pb", [L, E], f32, kind="ExternalInput")
    fwT_h = nc.dram_tensor("fwT", [L, E, 4 * E], bf16, kind="ExternalInput")
    fb_h = nc.dram_tensor("fb", [L, 4 * E], f32, kind="ExternalInput")
    owT_h = nc.dram_tensor("owT", [L, 4 * E, E], bf16, kind="ExternalInput")
    ob_h = nc.dram_tensor("ob", [L, E], f32, kind="ExternalInput")
    wteT_h = nc.dram_tensor("wteT", [E, VP], bf16, kind="ExternalInput")
    masks_h = nc.dram_tensor("masks", [P, 4, 8, P], bf16, kind="ExternalInput")

    logits_h = nc.dram_tensor("logits", [NMT * P, VP], f32,
                              kind="ExternalOutput")
    sumexp_h = nc.dram_tensor("sumexp", [NMT, P], f32, kind="ExternalOutput")

    pair_groups = [[0, 1], [2, 3], [4, 5], [6, 7]]
    all_groups = [list(range(NB))]

    with tile.TileContext(nc) as tc, ExitStack() as top:
        const = top.enter_context(tc.tile_pool(name="const", bufs=1))
        state = top.enter_context(tc.tile_pool(name="state", bufs=1))
        dram = top.enter_context(tc.tile_pool(name="dram", bufs=2,
                                              space="DRAM"))
        sm = top.enter_context(tc.tile_pool(name="sm", bufs=4))
        big_ps = top.enter_context(
            tc.tile_pool(name="big_ps", bufs=4, space="PSUM"))
        small_ps = top.enter_context(
            tc.tile_pool(name="small_ps", bufs=4, space="PSUM"))

        IDN = const.tile([P, P], bf16)
        make_identity(nc, IDN)
        EPS = const.tile([P, 1], f32)
        nc.vector.memset(EPS, 1e-5)
        MASKT = const.tile([P, 4, 8, P], bf16)
        nc.sync.dma_start(out=MASKT, in_=masks_h[:])

        X = state.tile([P, 4, E], f32)
        nc.sync.dma_start(out=X, in_=x0_h[:].rearrange("(j p) e -> p j e", p=P))

        def emit_ln(x_ap, out_ap):
            stats = sm.tile([P, 3, 6], f32, tag="bnst")
            for c in range(3):
                nc.vector.bn_stats(out=stats[:, c, :],
                                   in_=x_ap[:, c * 256:(c + 1) * 256])
            mv = sm.tile([P, 2], f32, tag="mv")
            nc.vector.bn_aggr(out=mv, in_=stats)
            std = sm.tile([P, 1], f32, tag="std")
            nc.scalar.activation(out=std, in_=mv[:, 1:2], func=AF.Sqrt,
                                 bias=EPS, scale=1.0)
            rstd = sm.tile([P, 1], f32, tag="rstd")
            nc.vector.reciprocal(out=rstd, in_=std)
            nc.vector.tensor_scalar(out=out_ap, in0=x_ap,
                                    scalar1=mv[:, 0:1], scalar2=rstd,
                                    op0=ALU.subtract, op1=ALU.mult)

        def emit_transpose_6(src_row_major, dst_hT, col_off):
            # src [128, 768] -> dst[:, et, col_off:col_off+128] for et in 0..5
            for et in range(ET):
                tp = small_ps.tile([P, P], bf16, tag="sm")
                nc.tensor.transpose(tp, src_row_major[:, et * P:(et + 1) * P],
                                    IDN)
                nc.scalar.activation(out=dst_hT[:, et, col_off:col_off + P],
                                     in_=tp, func=AF.Copy)

        with ExitStack() as trunk:
            wq = trunk.enter_context(tc.tile_pool(name="wq", bufs=3))
            wp = trunk.enter_context(tc.tile_pool(name="wp", bufs=2))
            wf = trunk.enter_context(tc.tile_pool(name="wf", bufs=3))
            wo = trunk.enter_context(tc.tile_pool(name="wo", bufs=3))
            bias_p = trunk.enter_context(tc.tile_pool(name="bias_p", bufs=2))
            h_pool = trunk.enter_context(tc.tile_pool(name="h_pool", bufs=2))
            hT_pool = trunk.enter_context(tc.tile_pool(name="hT_pool", bufs=2))
            qt_pool = trunk.enter_context(tc.tile_pool(name="qt_pool", bufs=1))
            kvt_pool = trunk.enter_context(tc.tile_pool(name="kvt_pool", bufs=1))
            kvg_pool = trunk.enter_context(tc.tile_pool(name="kvg_pool", bufs=1))
            vaug_pool = trunk.enter_context(tc.tile_pool(name="vaug_pool", bufs=1))
            yt_pool = trunk.enter_context(tc.tile_pool(name="yt_pool", bufs=1))
            gt_pool = trunk.enter_context(tc.tile_pool(name="gt_pool", bufs=1))
            expt_pool = trunk.enter_context(tc.tile_pool(name="expt_pool", bufs=4))
            y_pool = trunk.enter_context(tc.tile_pool(name="y_pool", bufs=3))

            for l in range(L):
                # ---- LN1 + transpose ----
                h1s = []
                for lt in range(4):
                    h1 = h_pool.tile([P, E], bf16, tag="h")
                    emit_ln(X[:, lt, :], h1)
                    h1s.append(h1)
                H1T = hT_pool.tile([P, ET, R], bf16, tag="hT")
                for lt in range(4):
                    emit_transpose_6(h1s[lt], H1T, lt * P)

                # ---- qkv projection (kv tiles first, then q) ----
                QT = qt_pool.tile([P, ET, R], bf16)
                KVT = kvt_pool.tile([P, 2 * ET, R], bf16)
                ab_t = bias_p.tile([P, OFT], f32, tag="ab")
                nc.sync.dma_start(
                    out=ab_t, in_=ab_h[l].rearrange("(o p) -> p o", p=P))
                for of in list(range(ET, OFT)) + list(range(ET)):
                    slab = wq.tile([P, ET, P], bf16, tag="wq")
                    nc.sync.dma_start(
                        out=slab,
                        in_=awT_h[l].rearrange("(et p) o -> p et o", p=P)
                        [:, :, ds(of * P, P)])
                    ps = big_ps.tile([P, R], f32, tag="big")
                    for et in range(ET):
                        nc.tensor.matmul(ps, slab[:, et, :], H1T[:, et, :],
                                         start=(et == 0), stop=(et == ET - 1))
                    dst = (KVT[:, of - ET, :] if of >= ET else QT[:, of, :])
                    nc.vector.tensor_scalar_add(out=dst, in0=ps,
                                                scalar1=ab_t[:, of:of + 1])

                # ---- K/V pair AllGather ----
                kv_in = dram.tile([2 * ET * P, R], bf16, tag="kv_in")
                nc.sync.dma_start(
                    out=kv_in[:].rearrange("(ot p) r -> p ot r", p=P), in_=KVT)
                kv_out = nc.dram_tensor([2 * 2 * ET * P, R], bf16,
                                        kind="Internal")
                nc.gpsimd.collective_compute(
                    "AllGather", mybir.AluOpType.bypass,
                    replica_groups=pair_groups,
                    ins=[kv_in[:].opt()], outs=[kv_out[:].opt()],
                )
                KVG = kvg_pool.tile([P, 2 * ET, 2 * R], bf16)
                for g in range(8):
                    blk, slot = PERM[g]
                    src = kv_out[blk * 2 * ET * P:(blk + 1) * 2 * ET * P, :]
                    nc.sync.dma_start(
                        out=KVG[:, :, ds(g * P, P)],
                        in_=src.rearrange("(ot p) r -> p ot r", p=P)
                        [:, :, ds(slot * P, P)])

                # ---- v_aug: transpose V to row-major + ones column ----
                VAUG = vaug_pool.tile([P, H, 8, D + 1], bf16)
                nc.vector.memset(VAUG[:, :, :, D:D + 1], 1.0)
                for ot in range(ET):          # v feature tiles (2 heads each)
                    for s in range(8):
                        tp = small_ps.tile([P, P], bf16, tag="sm")
                        nc.tensor.transpose(
                            tp, KVG[:, ET + ot, ds(s * P, P)], IDN)
                        nc.scalar.activation(
                            out=VAUG[:, 2 * ot, s, 0:D],
                            in_=tp[:, 0:D], func=AF.Copy)
                        nc.scalar.activation(
                            out=VAUG[:, 2 * ot + 1, s, 0:D],
                            in_=tp[:, D:2 * D], func=AF.Copy)

                # ---- attention ----
                YT = yt_pool.tile([P, ET, R], bf16)
                for j in range(4):
                    nk = NKPAD[j]
                    for h in range(H):
                        ro = (h % 2) * D
                        qa = QT[ro:ro + D, h // 2, ds(j * P, P)]
                        EXPT = expt_pool.tile([P, 8, P], bf16, tag="expt")
                        for s in range(nk):
                            st = small_ps.tile([P, P], f32, tag="sm")
                            ka = KVG[ro:ro + D, h // 2, ds(s * P, P)]
                            nc.tensor.matmul(st, ka, qa, start=True, stop=True)
                            nc.scalar.activation(out=EXPT[:, s, :], in_=st,
                                                 func=AF.Exp,
                                                 scale=1.0 / math.sqrt(D))
                            nc.vector.tensor_mul(EXPT[:, s, :], EXPT[:, s, :],
                                                 MASKT[:, j, s, :])
                        av = small_ps.tile([P, D + 1], f32, tag="sm")
                        for s in range(nk):
                            nc.tensor.matmul(av, EXPT[:, s, :],
                                             VAUG[:, h, s, :],
                                             start=(s == 0), stop=(s == nk - 1))
                        rec = sm.tile([P, 1], f32, tag="rec")
                        nc.vector.reciprocal(out=rec, in_=av[:, D:D + 1])
                        y_sb = y_pool.tile([P, D], bf16, tag="y")
                        nc.vector.tensor_scalar_mul(out=y_sb, in0=av[:, 0:D],
                                                    scalar1=rec)
                        yt_ps = small_ps.tile([D, P], bf16, tag="sm")
                        nc.tensor.transpose(yt_ps, y_sb, IDN)
                        nc.scalar.activation(
                            out=YT[ro:ro + D, h // 2, ds(j * P, P)],
                            in_=yt_ps, func=AF.Copy)

                # ---- attention out-projection + residual ----
                for nt, (n0, nsz) in enumerate([(0, 512), (512, 256)]):
                    slab = wp.tile([P, ET, 512], bf16, tag="wp")
                    nc.sync.dma_start(
                        out=slab[:, :, 0:nsz],
                        in_=pwT_h[l].rearrange("(et p) e -> p et e", p=P)
                        [:, :, ds(n0, nsz)])
                    for lt in range(4):
                        ps = big_ps.tile([P, nsz], f32, tag="big")
                        for et in range(ET):
                            nc.tensor.matmul(
                                ps, YT[:, et, ds(lt * P, P)],
                                slab[:, et, 0:nsz],
                                start=(et == 0), stop=(et == ET - 1))
                        nc.vector.tensor_add(X[:, lt, ds(n0, nsz)], ps,
                                             X[:, lt, ds(n0, nsz)])
                pbc = bias_p.tile([P, E], f32, tag="pbc")
                pb_l = pb_h[l]
                nc.sync.dma_start(out=pbc, in_=bass.AP(
                    tensor=pb_l.tensor, offset=pb_l.offset,
                    ap=[[0, P]] + [list(a) for a in pb_l.ap]))
                for lt in range(4):
                    nc.vector.tensor_add(X[:, lt, :], X[:, lt, :], pbc)

                # ---- LN2 + transpose ----
                h2s = []
                for lt in range(4):
                    h2 = h_pool.tile([P, E], bf16, tag="h")
                    emit_ln(X[:, lt, :], h2)
                    h2s.append(h2)
                H2T = hT_pool.tile([P, ET, R], bf16, tag="hT")
                for lt in range(4):
                    emit_transpose_6(h2s[lt], H2T, lt * P)

                # ---- FFN up + gelu (transposed output) ----
                GT = gt_pool.tile([P, FT, R], bf16)
                fb_t = bias_p.tile([P, FT], f32, tag="fb")
                nc.sync.dma_start(
                    out=fb_t, in_=fb_h[l].rearrange("(f p) -> p f", p=P))
                for ft in range(FT):
                    slab = wf.tile([P, ET, P], bf16, tag="wf")
                    nc.sync.dma_start(
                        out=slab,
                        in_=fwT_h[l].rearrange("(et p) f -> p et f", p=P)
                        [:, :, ds(ft * P, P)])
                    ps = big_ps.tile([P, R], f32, tag="big")
                    for et in range(ET):
                        nc.tensor.matmul(ps, slab[:, et, :], H2T[:, et, :],
                                         start=(et == 0), stop=(et == ET - 1))
                    nc.scalar.activation(out=GT[:, ft, :], in_=ps,
                                         func=AF.Gelu_apprx_tanh,
                                         bias=fb_t[:, ft:ft + 1], scale=1.0)

                # ---- FFN down + residual ----
                for nt, (n0, nsz) in enumerate([(0, 512), (512, 256)]):
                    pss = [big_ps.tile([P, nsz], f32, tag="big",
                                       name=f"ops_{l}_{nt}_{i}")
                           for i in range(4)]
                    for ft in range(FT):
                        slab = wo.tile([P, 512], bf16, tag="wo")
                        nc.sync.dma_start(
                            out=slab[:, 0:nsz],
                            in_=owT_h[l][ds(ft * P, P), ds(n0, nsz)])
                        for lt in range(4):
                            nc.tensor.matmul(
                                pss[lt], GT[:, ft, ds(lt * P, P)],
                                slab[:, 0:nsz],
                                start=(ft == 0), stop=(ft == FT - 1))
                    for lt in range(4):
                        nc.vector.tensor_add(X[:, lt, ds(n0, nsz)], pss[lt],
                                             X[:, lt, ds(n0, nsz)])
                obc = bias_p.tile([P, E], f32, tag="pbc")
                ob_l = ob_h[l]
                nc.sync.dma_start(out=obc, in_=bass.AP(
                    tensor=ob_l.tensor, offset=ob_l.offset,
                    ap=[[0, P]] + [list(a) for a in ob_l.ap]))
                for lt in range(4):
                    nc.vector.tensor_add(X[:, lt, :], X[:, lt, :], obc)

        # ---------------- lm head ----------------
        with ExitStack() as lm:
            h_pool = lm.enter_context(tc.tile_pool(name="lm_h", bufs=2))
            xft_pool = lm.enter_context(tc.tile_pool(name="lm_xft", bufs=1))
            xmt_pool = lm.enter_context(tc.tile_pool(name="lm_xmt", bufs=1))
            wv_pool = lm.enter_context(tc.tile_pool(name="lm_wv", bufs=3))
            se_pool = lm.enter_context(tc.tile_pool(name="lm_se", bufs=1))
            ex_pool = lm.enter_context(tc.tile_pool(name="lm_ex", bufs=3))
            lg_pool = lm.enter_context(tc.tile_pool(name="lm_lg", bufs=4))
            rd_pool = lm.enter_context(tc.tile_pool(name="lm_rd", bufs=4))

            XFT = xft_pool.tile([P, ET, R], bf16)
            for lt in range(4):
                xf = h_pool.tile([P, E], bf16, tag="xf")
                emit_ln(X[:, lt, :], xf)
                emit_transpose_6(xf, XFT, lt * P)

            xf_in = dram.tile([E, R], bf16, tag="xf_in")
            nc.sync.dma_start(
                out=xf_in[:].rearrange("(et p) r -> p et r", p=P), in_=XFT)
            xf_out = nc.dram_tensor([NB * E, R], bf16, kind="Internal",
                                    addr_space="Shared")
            nc.gpsimd.collective_compute(
                "AllGather", mybir.AluOpType.bypass,
                replica_groups=all_groups,
                ins=[xf_in[:].opt()], outs=[xf_out[:].opt()],
            )

            XMT = xmt_pool.tile([P, NMT, ET, P], bf16)
            for mt in range(NMT):
                cb, lt = mt // 4, mt % 4
                src = xf_out[cb * E:(cb + 1) * E, :]
                nc.sync.dma_start(
                    out=XMT[:, mt, :, :],
                    in_=src.rearrange("(et p) r -> p et r", p=P)
                    [:, :, ds(lt * P, P)])

            SEACC = se_pool.tile([P, NMT, NVT], f32)
            v0 = 0
            for vt in range(NVT):
                vsz = VT_SIZES[vt]
                slab = wv_pool.tile([P, ET, 512], bf16, tag="wv")
                nc.sync.dma_start(
                    out=slab[:, :, 0:vsz],
                    in_=wteT_h[:].rearrange("(et p) v -> p et v", p=P)
                    [:, :, ds(v0, vsz)])
                for mt in range(NMT):
                    ps = big_ps.tile([P, vsz], f32, tag="big")
                    for et in range(ET):
                        nc.tensor.matmul(ps, XMT[:, mt, et, :],
                                         slab[:, et, 0:vsz],
                                         start=(et == 0), stop=(et == ET - 1))
                    lg = lg_pool.tile([P, 512], f32, tag="lg")
                    nc.vector.tensor_copy(out=lg[:, 0:vsz], in_=ps)
                    nc.sync.dma_start(
                        out=logits_h[ds(mt * P, P), ds(v0, vsz)],
                        in_=lg[:, 0:vsz])
                    ex = ex_pool.tile([P, 512], bf16, tag="ex")
                    nc.scalar.activation(out=ex[:, 0:vsz], in_=ps, func=AF.Exp,
                                         accum_out=SEACC[:, mt, vt:vt + 1])
                v0 += vsz
            for mt in range(NMT):
                red = rd_pool.tile([P, 1], f32, tag="red")
                nc.vector.reduce_sum(out=red, in_=SEACC[:, mt, :],
                                     axis=mybir.AxisListType.X)
                nc.sync.dma_start(out=sumexp_h[mt], in_=red)

    import concourse.mybir as mybir_mod
    _split_sync_waits(nc, mybir_mod)
    return nc


# ---------------------------------------------------------------- host side
def _host_prepare(inp):
    """Fold LN affines into weights, build per-core input maps."""
    f32 = np.float32
    wte = np.asarray(inp["wte"], f32)
    wpe = np.asarray(inp["wpe"], f32)
    idx = np.asarray(inp["idx"])
    ln1_w = np.asarray(inp["ln1_w"], f32); ln1_b = np.asarray(inp["ln1_b"], f32)
    ln2_w = np.asarray(inp["ln2_w"], f32); ln2_b = np.asarray(inp["ln2_b"], f32)
    lnf_w = np.asarray(inp["lnf_w"], f32); lnf_b = np.asarray(inp["lnf_b"], f32)
    attn_w = np.asarray(inp["attn_w"], f32); attn_b = np.asarray(inp["attn_b"], f32)
    proj_w = np.asarray(inp["proj_w"], f32); proj_b = np.asarray(inp["proj_b"], f32)
    fc_w = np.asarray(inp["fc_w"], f32); fc_b = np.asarray(inp["fc_b"], f32)
    out_w = np.asarray(inp["out_w"], f32); out_b = np.asarray(inp["out_b"], f32)

    aw_f = attn_w * ln1_w[:, None, :]
    ab_f = attn_b + np.einsum("loe,le->lo", attn_w, ln1_b)
    fw_f = fc_w * ln2_w[:, None, :]
    fb_f = fc_b + np.einsum("lfe,le->lf", fc_w, ln2_b)
    awT = np.ascontiguousarray(aw_f.transpose(0, 2, 1)).astype(BF16)
    pwT = np.ascontiguousarray(proj_w.transpose(0, 2, 1)).astype(BF16)
    fwT = np.ascontiguousarray(fw_f.transpose(0, 2, 1)).astype(BF16)
    owT = np.ascontiguousarray(out_w.transpose(0, 2, 1)).astype(BF16)

    wte_f = wte * lnf_w[None, :]
    wteT_full = np.zeros((E, NB * VP), BF16)
    wteT_full[:, :V] = wte_f.T.astype(BF16)
    logit_bias = wte @ lnf_b       # nonzero only if lnf_b != 0

    kr = np.arange(P)
    qr = np.arange(P)
    masks = {}
    for par in (0, 1):
        m = np.zeros((P, 4, 8, P), np.float32)
        for j in range(4):
            qi = ZIG[par][j]
            for s in range(8):
                m[:, j, s, :] = ((s * P + kr)[:, None]
                                 <= (qi * P + qr)[None, :])
        masks[par] = m.astype(BF16)

    pos_emb = wpe[:T]
    in_maps = []
    for c in range(NB):
        b = c // 2
        par = c % 2
        rows_x0 = np.empty((R, E), f32)
        for j, gt in enumerate(ZIG[par]):
            t0 = gt * P
            rows_x0[j * P:(j + 1) * P] = wte[idx[b, t0:t0 + P]] \
                + pos_emb[t0:t0 + P]
        in_maps.append({
            "x0": rows_x0,
            "awT": awT, "ab": ab_f.astype(f32),
            "pwT": pwT, "pb": proj_b.astype(f32),
            "fwT": fwT, "fb": fb_f.astype(f32),
            "owT": owT, "ob": out_b.astype(f32),
            "wteT": np.ascontiguousarray(wteT_full[:, c * VP:(c + 1) * VP]),
            "masks": masks[par],
        })
    return in_maps, logit_bias


def _assemble(results, targets, logit_bias):
    f32 = np.float32
    full_int = np.concatenate([r["logits"] for r in results], axis=1)[:, :V]
    logits = np.empty((B, T, V), f32)
    lse_bt = np.empty((B, T), f32)
    sumexp_tot = np.zeros(NMT * P, f32)
    for r in results:
        sumexp_tot += r["sumexp"].reshape(-1)
    sumexp_tot -= float(VPAD)          # padded cols contribute exp(0)=1 each
    blk = 0
    for c in range(NB):
        b = c // 2
        for gt in ZIG[c % 2]:
            logits[b, gt * P:(gt + 1) * P] = full_int[blk * P:(blk + 1) * P]
            lse_bt[b, gt * P:(gt + 1) * P] = np.log(
                sumexp_tot[blk * P:(blk + 1) * P])
            blk += 1
    if np.any(logit_bias != 0.0):
        logits += logit_bias[None, None, :].astype(f32)
        # lse would need recomputation in this (unused for this data) case
        lse_bt = np.log(np.exp(logits.astype(np.float64)).sum(-1)).astype(f32)
    tgt = np.asarray(targets)
    tl = np.take_along_axis(logits[:, :-1], tgt[:, 1:][..., None],
                            axis=-1)[..., 0]
    loss = f32((lse_bt[:, :-1] - tl).mean())
    return logits, loss


_RUN_KWARGS = {}   # test harness may set e.g. {"trace": True}


def kernel(**inputs):
    from concourse.bass_utils import run_bass_kernel_spmd

    if "nc" not in _CACHE:
        _CACHE["nc"] = _build_program()
    nc = _CACHE["nc"]

    in_maps, logit_bias = _host_prepare(inputs)
    res = run_bass_kernel_spmd(nc, in_maps, core_ids=list(range(NB)),
                               **_RUN_KWARGS)
    _CACHE["last_result"] = res
    logits, loss = _assemble(res.results, inputs["targets"], logit_bias)
    return logits, loss
